# revision 1
# baseline (speedup 1.0000x reference)
"""Bone_Direction_GCN fused kernel for 8 Trainium2 NeuronCores.

Data-parallel over the batch dim: each core processes 2048 of 16384 batches.
All graph mixing (GCN conv + dense-adj einsum) is expressed as block-diagonal
matmuls over groups of 7 batches (7*17 = 119 rows <= 128 partitions), fully
fused with the channel matmuls on the PE array in bf16 (fp32 residual).
"""

import sys

sys.path.insert(0, "/opt/trn_rl_repo")

import numpy as np
import ml_dtypes

B, J, E = 16384, 17, 32
CIN, COUT = 128, 128
MID = COUT // 2
PROP = 0.5
SLOPE = 0.01

N_CORES = 8
BC = B // N_CORES          # batches per core (2048)
ROWS = BC * J              # rows per core (34816)
G = 7                      # batches per sub-tile
R = G * J                  # rows per sub-tile (119)
S = 4                      # sub-tiles per macro-tile
RM = S * R                 # rows per macro-tile (476)
NM = 73                    # macro tiles per core (73*476 = 34748)
GT = BC - NM * S * G       # tail batches (4)
RT = GT * J                # tail rows (68)

assert NM * RM + RT == ROWS

_CACHE = {}


def _gcn_matrix(edge_index: np.ndarray, edge_weight: np.ndarray) -> np.ndarray:
    """Dense normalized GCN operator M with out[i] = sum_j M[i, j] * x[j]."""
    row = edge_index[0].astype(np.int64)
    col = edge_index[1].astype(np.int64)
    loop = np.arange(J, dtype=np.int64)
    row_f = np.concatenate([row, loop])
    col_f = np.concatenate([col, loop])
    w_f = np.concatenate([edge_weight.astype(np.float32), np.ones(J, np.float32)])
    deg = np.zeros(J, np.float32)
    np.add.at(deg, col_f, w_f)
    safe = np.where(deg > 0, deg, 1.0).astype(np.float32)
    dis = np.where(deg > 0, 1.0 / np.sqrt(safe), 0.0).astype(np.float32)
    norm = dis[row_f] * w_f * dis[col_f]
    M = np.zeros((J, J), np.float32)
    np.add.at(M, (col_f, row_f), norm)
    return M


def _block_diag(block: np.ndarray, n: int) -> np.ndarray:
    j = block.shape[0]
    out = np.zeros((n * j, n * j), block.dtype)
    for g in range(n):
        out[g * j:(g + 1) * j, g * j:(g + 1) * j] = block
    return out


def _mix_consts(M: np.ndarray, adj: np.ndarray, g: int):
    """mixI [g*17, 2*g*17] = [blockdiag(M.T) | I]; mix2e [g*17+2, g*17]."""
    r = g * J
    mix1 = _block_diag(M.T, g)
    mixI = np.concatenate([mix1, np.eye(r, dtype=np.float32)], axis=1)
    mix2 = _block_diag(PROP * adj, g)
    ones_row = np.ones((1, r), np.float32)
    s_row = np.tile(PROP * adj.sum(axis=0), g)[None, :]
    mix2e = np.concatenate([mix2, ones_row, s_row], axis=0)
    return mixI, mix2e


def _build_bass(leaky_mode: str = "lrelu", **_ignored):
    import concourse.bacc as bacc
    import concourse.mybir as mybir
    import concourse.tile as tile
    from contextlib import ExitStack

    f32 = mybir.dt.float32
    bf16 = mybir.dt.bfloat16

    nc = bacc.Bacc("TRN2", target_bir_lowering=False, debug=False)

    x_d = nc.dram_tensor("x", [ROWS, CIN], f32, kind="ExternalInput").ap()
    mixI_d = nc.dram_tensor("mixI", [R, 2 * R], bf16, kind="ExternalInput").ap()
    mix2e_d = nc.dram_tensor("mix2e", [R + 2, R], bf16, kind="ExternalInput").ap()
    mixIt_d = nc.dram_tensor("mixIt", [RT, 2 * RT], bf16, kind="ExternalInput").ap()
    mix2et_d = nc.dram_tensor("mix2et", [RT + 2, RT], bf16, kind="ExternalInput").ap()
    w1_d = nc.dram_tensor("w1", [CIN, COUT], bf16, kind="ExternalInput").ap()
    w2t_d = nc.dram_tensor("w2t", [CIN, MID], bf16, kind="ExternalInput").ap()
    w4t_d = nc.dram_tensor("w4t", [MID, COUT], bf16, kind="ExternalInput").ap()
    b2_d = nc.dram_tensor("b2", [MID, 1], f32, kind="ExternalInput").ap()
    ab2_d = nc.dram_tensor("ab2", [MID, 1], f32, kind="ExternalInput").ap()
    b1b4_d = nc.dram_tensor("b1b4", [2, S * COUT], bf16, kind="ExternalInput").ap()
    o_d = nc.dram_tensor("out", [ROWS, CIN], f32, kind="ExternalOutput").ap()

    with ExitStack() as ctx:
        tc = ctx.enter_context(tile.TileContext(nc))

        const = ctx.enter_context(tc.tile_pool(name="const", bufs=1))
        mixI_sb = const.tile_from(mixI_d)
        mix2e_sb = const.tile_from(mix2e_d)
        mixIt_sb = const.tile_from(mixIt_d)
        mix2et_sb = const.tile_from(mix2et_d)
        w1_sb = const.tile_from(w1_d)
        w2t_sb = const.tile_from(w2t_d)
        w4t_sb = const.tile_from(w4t_d)
        b2_sb = const.tile_from(b2_d)
        ab2_sb = const.tile_from(ab2_d)

        def leaky(hbf, psH):
            if leaky_mode == "lrelu":
                nc.scalar.activation(
                    hbf[:], psH[:],
                    func=mybir.ActivationFunctionType.Lrelu,
                    bias=b2_sb[:], scale=1.0, alpha=SLOPE,
                )
            else:
                a = h_pool.tile(list(psH.shape), bf16, tag="lk_a")
                nc.scalar.activation(
                    a[:], psH[:],
                    func=mybir.ActivationFunctionType.Identity,
                    bias=ab2_sb[:], scale=SLOPE,
                )
                nc.vector.scalar_tensor_tensor(
                    hbf[:], psH[:], b2_sb[:], a[:],
                    op0=mybir.AluOpType.add, op1=mybir.AluOpType.max,
                )

        y2e_pool = ctx.enter_context(tc.tile_pool(name="y2e", bufs=2))
        y2e_tiles = []
        for i in range(2):
            t = y2e_pool.tile([R + 2, S * COUT], bf16, tag=f"y2e{i}")
            nc.sync.dma_start(out=t[R:R + 2, :], in_=b1b4_d)
            y2e_tiles.append(t)
        y2et_pool = ctx.enter_context(tc.tile_pool(name="y2et", bufs=1))
        y2et = y2et_pool.tile([RT + 2, COUT], bf16)
        nc.sync.dma_start(out=y2et[RT:RT + 2, :], in_=b1b4_d[:, 0:COUT])

        xin_pool = ctx.enter_context(tc.tile_pool(name="xin", bufs=3))
        xbf_pool = ctx.enter_context(tc.tile_pool(name="xbf", bufs=3))
        xm_pool = ctx.enter_context(tc.tile_pool(name="xm", bufs=2))
        xt_pool = ctx.enter_context(tc.tile_pool(name="xt", bufs=2))
        h_pool = ctx.enter_context(tc.tile_pool(name="h", bufs=2))
        out_pool = ctx.enter_context(tc.tile_pool(name="osb", bufs=3))

        psT_pool = ctx.enter_context(tc.tile_pool(name="psT", bufs=1, space="PSUM"))
        psH_pool = ctx.enter_context(tc.tile_pool(name="psH", bufs=2, space="PSUM"))
        psY2_pool = ctx.enter_context(tc.tile_pool(name="psY2", bufs=1, space="PSUM"))
        psO_pool = ctx.enter_context(tc.tile_pool(name="psO", bufs=1, space="PSUM"))

        for m in range(NM):
            r0 = m * RM
            xin = xin_pool.tile([R, S * CIN], f32)
            nc.sync.dma_start(
                out=xin[:].rearrange("p (s c) -> p s c", c=CIN),
                in_=x_d[r0:r0 + RM, :].rearrange("(s p) c -> p s c", p=R),
            )
            xbf = xbf_pool.tile([R, S * CIN], bf16)
            nc.gpsimd.tensor_copy(xbf[:], xin[:])

            xm = xm_pool.tile([CIN, S * R], bf16)
            xt = xt_pool.tile([CIN, S * R], bf16)
            psT = psT_pool.tile([CIN, S * 512], f32)
            for s in range(S):
                nc.tensor.matmul(
                    psT[:, s * 512:s * 512 + 2 * R],
                    lhsT=xbf[:, s * CIN:(s + 1) * CIN],
                    rhs=mixI_sb[:],
                    start=True, stop=True,
                )
                nc.vector.tensor_copy(
                    xm[:, s * R:(s + 1) * R], psT[:, s * 512:s * 512 + R])
                nc.scalar.copy(
                    xt[:, s * R:(s + 1) * R], psT[:, s * 512 + R:s * 512 + 2 * R])

            psH = psH_pool.tile([MID, RM], f32)
            for s in range(S):
                nc.tensor.matmul(
                    psH[:, s * R:(s + 1) * R],
                    lhsT=w2t_sb[:], rhs=xt[:, s * R:(s + 1) * R],
                    start=True, stop=True,
                )
            hbf = h_pool.tile([MID, RM], bf16)
            leaky(hbf, psH)
            psY2 = psY2_pool.tile([R, S * COUT], f32)
            for s in range(S):
                nc.tensor.matmul(
                    psY2[:, s * COUT:(s + 1) * COUT],
                    lhsT=hbf[:, s * R:(s + 1) * R], rhs=w4t_sb[:],
                    start=True, stop=True,
                )
            y2e = y2e_tiles[m % 2]
            nc.scalar.copy(y2e[0:R, :], psY2[:])

            psO = psO_pool.tile([R, S * COUT], f32)
            for s in range(S):
                nc.tensor.matmul(
                    psO[:, s * COUT:(s + 1) * COUT],
                    lhsT=xm[:, s * R:(s + 1) * R], rhs=w1_sb[:],
                    start=True, stop=False, skip_group_check=True,
                )
                nc.tensor.matmul(
                    psO[:, s * COUT:(s + 1) * COUT],
                    lhsT=mix2e_sb[:], rhs=y2e[:, s * COUT:(s + 1) * COUT],
                    start=False, stop=True, skip_group_check=True,
                )
            out_sb = out_pool.tile([R, S * CIN], f32)
            nc.vector.tensor_add(out_sb[:], psO[:], xin[:])
            nc.sync.dma_start(
                out=o_d[r0:r0 + RM, :].rearrange("(s p) c -> p s c", p=R),
                in_=out_sb[:].rearrange("p (s c) -> p s c", c=CIN),
            )

        r0 = NM * RM
        xin = xin_pool.tile([RT, CIN], f32, tag="xin")
        nc.sync.dma_start(out=xin[:], in_=x_d[r0:r0 + RT, :])
        xbf = xbf_pool.tile([RT, CIN], bf16, tag="xbf")
        nc.gpsimd.tensor_copy(xbf[:], xin[:])
        psT = psT_pool.tile([CIN, 2 * RT], f32, tag="psT")
        nc.tensor.matmul(psT[:], lhsT=xbf[:], rhs=mixIt_sb[:], start=True, stop=True)
        xm = xm_pool.tile([CIN, RT], bf16, tag="xm")
        nc.vector.tensor_copy(xm[:], psT[:, 0:RT])
        xt = xt_pool.tile([CIN, RT], bf16, tag="xt")
        nc.scalar.copy(xt[:], psT[:, RT:2 * RT])
        psH = psH_pool.tile([MID, RT], f32, tag="psH")
        nc.tensor.matmul(psH[:], lhsT=w2t_sb[:], rhs=xt[:], start=True, stop=True)
        hbf = h_pool.tile([MID, RT], bf16, tag="hbf")
        leaky(hbf, psH)
        psY2 = psY2_pool.tile([RT, COUT], f32, tag="psY2")
        nc.tensor.matmul(psY2[:], lhsT=hbf[:], rhs=w4t_sb[:], start=True, stop=True)
        nc.scalar.copy(y2et[0:RT, :], psY2[:])
        psO = psO_pool.tile([RT, COUT], f32, tag="psO")
        nc.tensor.matmul(psO[:], lhsT=xm[:], rhs=w1_sb[:],
                         start=True, stop=False, skip_group_check=True)
        nc.tensor.matmul(psO[:], lhsT=mix2et_sb[:], rhs=y2et[:],
                         start=False, stop=True, skip_group_check=True)
        out_sb = out_pool.tile([RT, CIN], f32, tag="out_sb")
        nc.vector.tensor_add(out_sb[:], psO[:], xin[:])
        nc.sync.dma_start(out=o_d[r0:r0 + RT, :], in_=out_sb[:])

    nc.compile()
    return nc


def _host_consts(inputs):
    bf = ml_dtypes.bfloat16
    M = _gcn_matrix(np.asarray(inputs["edge_index"]), np.asarray(inputs["edge_weight"]))
    adj = np.asarray(inputs["adj"], np.float32)
    mixI, mix2e = _mix_consts(M, adj, G)
    mixIt, mix2et = _mix_consts(M, adj, GT)
    W1 = np.asarray(inputs["W1"], np.float32)
    W2 = np.asarray(inputs["W2"], np.float32)
    W4 = np.asarray(inputs["W4"], np.float32)
    b1 = np.asarray(inputs["b1"], np.float32)
    b2 = np.asarray(inputs["b2"], np.float32)
    b4 = np.asarray(inputs["b4"], np.float32)
    b1b4 = np.stack([np.tile(b1, S), np.tile(b4, S)])
    return {
        "mixI": mixI.astype(bf),
        "mix2e": mix2e.astype(bf),
        "mixIt": mixIt.astype(bf),
        "mix2et": mix2et.astype(bf),
        "w1": np.ascontiguousarray(W1).astype(bf),
        "w2t": np.ascontiguousarray(W2.T).astype(bf),
        "w4t": np.ascontiguousarray(W4.T).astype(bf),
        "b2": np.ascontiguousarray(b2[:, None]),
        "ab2": np.ascontiguousarray(SLOPE * b2[:, None]),
        "b1b4": b1b4.astype(bf),
    }


def kernel(**inputs) -> np.ndarray:
    from concourse.bass_utils import run_bass_kernel_spmd

    if "nc" not in _CACHE:
        _CACHE["nc"] = _build_bass()
    nc = _CACHE["nc"]

    consts = _host_consts(inputs)
    vector = np.ascontiguousarray(np.asarray(inputs["vector"], np.float32))
    in_maps = []
    for c in range(N_CORES):
        m = dict(consts)
        m["x"] = np.ascontiguousarray(
            vector[c * BC:(c + 1) * BC].reshape(ROWS, CIN)
        )
        in_maps.append(m)

    res = run_bass_kernel_spmd(nc, in_maps, core_ids=list(range(N_CORES)))
    outs = [res.results[c]["out"].reshape(BC, J, CIN) for c in range(N_CORES)]
    return np.concatenate(outs, axis=0)



# revision 6
# speedup vs baseline: 1.8001x; 1.8001x over previous
"""Bone_Direction_GCN fused kernel for 8 Trainium2 NeuronCores.

Data-parallel over the batch dim: each core processes 2048 of 16384 batches.
All graph mixing (GCN conv + dense-adj einsum) is expressed as block-diagonal
matmuls over groups of 7 batches (7*17 = 119 rows <= 128 partitions), fully
fused with the channel matmuls on the PE array in bf16 (fp32 residual).

v2: software-pipelined macro-tile loop (PE never waits on same-tile
evacuations), DMAs batched 4 macro-tiles at a time (1904 rows per transfer),
PSUM packed 2 sub-tiles per bank, PSUM-evacuation work split across DVE and
ACT to balance engine load.
"""

import sys

sys.path.insert(0, "/opt/trn_rl_repo")

import numpy as np
import ml_dtypes

B, J, E = 16384, 17, 32
CIN, COUT = 128, 128
MID = COUT // 2
PROP = 0.5
SLOPE = 0.01

N_CORES = 8
BC = B // N_CORES          # batches per core (2048)
ROWS = BC * J              # rows per core (34816)
G = 7                      # batches per sub-tile
R = G * J                  # rows per sub-tile (119)
S = 4                      # sub-tiles per macro-tile
RM = S * R                 # rows per macro-tile (476)
LM = 4                     # macro-tiles per DMA group
NG = 18                    # full DMA groups (18*4 = 72 macro-tiles)
NMM = NG * LM              # pipelined macro-tiles (72)
NM = 73                    # total macro-tiles per core (73*476 = 34748)
GT = BC - NM * S * G       # tail batches (4)
RT = GT * J                # tail rows (68)
CPAD = 256                 # psT column stride per sub-tile (18 pad f32 cols)

assert NM * RM + RT == ROWS

_CACHE = {}


def _gcn_matrix(edge_index: np.ndarray, edge_weight: np.ndarray) -> np.ndarray:
    """Dense normalized GCN operator M with out[i] = sum_j M[i, j] * x[j]."""
    row = edge_index[0].astype(np.int64)
    col = edge_index[1].astype(np.int64)
    loop = np.arange(J, dtype=np.int64)
    row_f = np.concatenate([row, loop])
    col_f = np.concatenate([col, loop])
    w_f = np.concatenate([edge_weight.astype(np.float32), np.ones(J, np.float32)])
    deg = np.zeros(J, np.float32)
    np.add.at(deg, col_f, w_f)
    safe = np.where(deg > 0, deg, 1.0).astype(np.float32)
    dis = np.where(deg > 0, 1.0 / np.sqrt(safe), 0.0).astype(np.float32)
    norm = dis[row_f] * w_f * dis[col_f]
    M = np.zeros((J, J), np.float32)
    np.add.at(M, (col_f, row_f), norm)
    return M


def _block_diag(block: np.ndarray, n: int) -> np.ndarray:
    j = block.shape[0]
    out = np.zeros((n * j, n * j), block.dtype)
    for g in range(n):
        out[g * j:(g + 1) * j, g * j:(g + 1) * j] = block
    return out


def _mix_consts(M: np.ndarray, adj: np.ndarray, g: int):
    """mixI [g*17, 2*g*17] = [blockdiag(M.T) | I]; mix2e [g*17+2, g*17]."""
    r = g * J
    mix1 = _block_diag(M.T, g)
    mixI = np.concatenate([mix1, np.eye(r, dtype=np.float32)], axis=1)
    mix2 = _block_diag(PROP * adj, g)
    ones_row = np.ones((1, r), np.float32)
    s_row = np.tile(PROP * adj.sum(axis=0), g)[None, :]
    mix2e = np.concatenate([mix2, ones_row, s_row], axis=0)
    return mixI, mix2e


def _build_bass(leaky_mode: str = "lrelu", **_ignored):
    import concourse.bacc as bacc
    import concourse.mybir as mybir
    import concourse.tile as tile
    from contextlib import ExitStack

    f32 = mybir.dt.float32
    bf16 = mybir.dt.bfloat16

    nc = bacc.Bacc("TRN2", target_bir_lowering=False, debug=False)

    x_d = nc.dram_tensor("x", [ROWS, CIN], f32, kind="ExternalInput").ap()
    mixI_d = nc.dram_tensor("mixI", [R, CPAD], bf16, kind="ExternalInput").ap()
    mix2e_d = nc.dram_tensor("mix2e", [R + 2, R], bf16, kind="ExternalInput").ap()
    mixIt_d = nc.dram_tensor("mixIt", [RT, 2 * RT], bf16, kind="ExternalInput").ap()
    mix2et_d = nc.dram_tensor("mix2et", [RT + 2, RT], bf16, kind="ExternalInput").ap()
    w1_d = nc.dram_tensor("w1", [CIN, COUT], bf16, kind="ExternalInput").ap()
    w2t_d = nc.dram_tensor("w2t", [CIN, MID], bf16, kind="ExternalInput").ap()
    w4t_d = nc.dram_tensor("w4t", [MID, COUT], bf16, kind="ExternalInput").ap()
    b2_d = nc.dram_tensor("b2", [MID, 1], f32, kind="ExternalInput").ap()
    ab2_d = nc.dram_tensor("ab2", [MID, 1], f32, kind="ExternalInput").ap()
    b1b4_d = nc.dram_tensor("b1b4", [2, S * COUT], bf16, kind="ExternalInput").ap()
    o_d = nc.dram_tensor("out", [ROWS, CIN], f32, kind="ExternalOutput").ap()

    with ExitStack() as ctx:
        tc = ctx.enter_context(tile.TileContext(nc))

        const = ctx.enter_context(tc.tile_pool(name="const", bufs=1))
        mixI_sb = const.tile_from(mixI_d)
        mix2e_sb = const.tile_from(mix2e_d)
        mixIt_sb = const.tile_from(mixIt_d)
        mix2et_sb = const.tile_from(mix2et_d)
        w1_sb = const.tile_from(w1_d)
        w2t_sb = const.tile_from(w2t_d)
        w4t_sb = const.tile_from(w4t_d)
        b2_sb = const.tile_from(b2_d)
        ab2_sb = const.tile_from(ab2_d)

        def leaky(hbf, psH):
            if leaky_mode == "lrelu":
                nc.scalar.activation(
                    hbf[:], psH[:],
                    func=mybir.ActivationFunctionType.Lrelu,
                    bias=b2_sb[:], scale=1.0, alpha=SLOPE,
                )
            else:
                a = lk_pool.tile(list(psH.shape), bf16, tag="lk_a", name="lk_a")
                nc.scalar.activation(
                    a[:], psH[:],
                    func=mybir.ActivationFunctionType.Identity,
                    bias=ab2_sb[:], scale=SLOPE,
                )
                nc.vector.scalar_tensor_tensor(
                    hbf[:], psH[:], b2_sb[:], a[:],
                    op0=mybir.AluOpType.add, op1=mybir.AluOpType.max,
                )

        # y2e tiles: rows 0:R hold branch-2 matmul output d; rows R..R+2 hold
        # the (b1, b4) bias rows, preloaded once and reused every macro-tile.
        y2e_pool = ctx.enter_context(tc.tile_pool(name="y2e", bufs=2))
        y2e_tiles = []
        for i in range(2):
            t = y2e_pool.tile([R + 2, S * COUT], bf16, tag=f"y2e{i}")
            nc.sync.dma_start(out=t[R:R + 2, :], in_=b1b4_d)
            y2e_tiles.append(t)
        y2et_pool = ctx.enter_context(tc.tile_pool(name="y2et", bufs=1))
        y2et = y2et_pool.tile([RT + 2, COUT], bf16)
        nc.sync.dma_start(out=y2et[RT:RT + 2, :], in_=b1b4_d[:, 0:COUT])

        lk_pool = ctx.enter_context(tc.tile_pool(name="lk", bufs=2))

        # group-sized staging tiles (LM macro-tiles = 1904 rows per DMA)
        xin_pool = ctx.enter_context(tc.tile_pool(name="xin", bufs=2))
        ogrp_pool = ctx.enter_context(tc.tile_pool(name="ogrp", bufs=2))
        xbf_pool = ctx.enter_context(tc.tile_pool(name="xbf", bufs=3))
        xmxt_pool = ctx.enter_context(tc.tile_pool(name="xmxt", bufs=3))
        h_pool = ctx.enter_context(tc.tile_pool(name="h", bufs=2))

        psT_pool = ctx.enter_context(tc.tile_pool(name="psT", bufs=2, space="PSUM"))
        psH_pool = ctx.enter_context(tc.tile_pool(name="psH", bufs=1, space="PSUM"))
        psY2_pool = ctx.enter_context(tc.tile_pool(name="psY2", bufs=1, space="PSUM"))
        psO_pool = ctx.enter_context(tc.tile_pool(name="psO", bufs=2, space="PSUM"))

        xin_tiles = [None] * (NG + 1)
        ogrp_tiles = [None] * (NG + 1)
        xbf_tiles = [None] * NMM
        xmxt_tiles = [None] * NMM
        psT_tiles = [None] * NMM
        psH_tiles = [None] * NMM
        psY2_tiles = [None] * NMM
        psO_tiles = [None] * NMM
        hbf_tiles = [None] * NMM

        def xin_slice(m):
            g, k = divmod(m, LM)
            return xin_tiles[g][:, k * S * CIN:(k + 1) * S * CIN]

        def stage_load(g):
            r0 = g * LM * RM
            t = xin_pool.tile([R, LM * S * CIN], f32, tag="xin", name="xin")
            xin_tiles[g] = t
            nc.sync.dma_start(
                out=t[:].rearrange("p (q c) -> p q c", c=CIN),
                in_=x_d[r0:r0 + LM * RM, :].rearrange("(q p) c -> p q c", p=R),
            )
            ogrp_tiles[g] = ogrp_pool.tile([R, LM * S * CIN], f32, tag="ogrp", name="ogrp")

        def stage_cast(m):
            t = xbf_pool.tile([R, S * CIN], bf16, tag="xbf", name="xbf")
            xbf_tiles[m] = t
            nc.gpsimd.tensor_copy(t[:], xin_slice(m))

        def stage_mix(m):
            xbf = xbf_tiles[m]
            psT = psT_pool.tile([CIN, S * CPAD], f32, tag="psT", name="psT")
            psT_tiles[m] = psT
            for s in range(S):
                nc.tensor.matmul(
                    psT[:, s * CPAD:(s + 1) * CPAD],
                    lhsT=xbf[:, s * CIN:(s + 1) * CIN],
                    rhs=mixI_sb[:],
                    start=True, stop=True,
                )

        def stage_copies(m):
            psT = psT_tiles[m]
            xmxt = xmxt_pool.tile([CIN, S * CPAD], bf16, tag="xmxt", name="xmxt")
            xmxt_tiles[m] = xmxt
            half = CPAD + CPAD // 2
            nc.vector.tensor_copy(xmxt[:, 0:half], psT[:, 0:half])
            nc.scalar.copy(xmxt[:, half:], psT[:, half:])

        def stage_w2(m):
            xmxt = xmxt_tiles[m]
            xt = xmxt[:].rearrange("p (s q) -> p s q", q=CPAD)[:, :, R:2 * R]
            psH = psH_pool.tile([MID, RM], f32, tag="psH", name="psH")
            psH_tiles[m] = psH
            nc.tensor.matmul(psH[:], lhsT=w2t_sb[:], rhs=xt, start=True, stop=True)
            hbf = h_pool.tile([MID, RM], bf16, tag="hbf", name="hbf")
            hbf_tiles[m] = hbf
            leaky(hbf, psH)

        def stage_w4(m):
            hbf = hbf_tiles[m]
            psY2 = psY2_pool.tile([R, S * COUT], f32, tag="psY2", name="psY2")
            psY2_tiles[m] = psY2
            for s in range(S):
                nc.tensor.matmul(
                    psY2[:, s * COUT:(s + 1) * COUT],
                    lhsT=hbf[:, s * R:(s + 1) * R], rhs=w4t_sb[:],
                    start=True, stop=True,
                )

        def stage_y2e(m):
            psY2 = psY2_tiles[m]
            y2e = y2e_tiles[m % 2]
            nc.vector.tensor_copy(y2e[0:R, 0:2 * COUT], psY2[:, 0:2 * COUT])
            nc.scalar.copy(y2e[0:R, 2 * COUT:], psY2[:, 2 * COUT:])

        def stage_out(m):
            xmxt = xmxt_tiles[m]
            y2e = y2e_tiles[m % 2]
            psO = psO_pool.tile([R, S * COUT], f32, tag="psO", name="psO")
            psO_tiles[m] = psO
            for s in range(S):
                nc.tensor.matmul(
                    psO[:, s * COUT:(s + 1) * COUT],
                    lhsT=xmxt[:, s * CPAD:s * CPAD + R], rhs=w1_sb[:],
                    start=True, stop=False, skip_group_check=True,
                )
                nc.tensor.matmul(
                    psO[:, s * COUT:(s + 1) * COUT],
                    lhsT=mix2e_sb[:], rhs=y2e[:, s * COUT:(s + 1) * COUT],
                    start=False, stop=True, skip_group_check=True,
                )

        def stage_add(m):
            g, k = divmod(m, LM)
            ogrp = ogrp_tiles[g]
            nc.vector.tensor_add(
                ogrp[:, k * S * CIN:(k + 1) * S * CIN], psO_tiles[m][:], xin_slice(m))

        def stage_store(g):
            r0 = g * LM * RM
            nc.sync.dma_start(
                out=o_d[r0:r0 + LM * RM, :].rearrange("(q p) c -> p q c", p=R),
                in_=ogrp_tiles[g][:].rearrange("p (q c) -> p q c", c=CIN),
            )

        # software-pipelined main loop; stage offsets chosen so PE never waits
        # on the current iteration's DVE/ACT evacuations.
        for it in range(NMM + 3):
            m1, m2, m3 = it - 1, it - 2, it - 3
            if it < NMM and it % LM == 0:
                stage_load(it // LM)
            if it < NMM:
                stage_cast(it)
            if 0 <= m3 < NMM:
                stage_y2e(m3)       # DVE/ACT: ready at iteration start
            if 0 <= m1 < NMM:
                stage_mix(m1)       # PE
            if 0 <= m2 < NMM:
                stage_w2(m2)        # PE + ACT leaky (before the big copyB)
            if 0 <= m1 < NMM:
                stage_copies(m1)    # DVE/ACT psT evacuation
            if 0 <= m3 < NMM:
                stage_out(m3)       # PE W1+mix2e
                stage_add(m3)       # DVE, slack-tolerant
            if 0 <= m2 < NMM:
                stage_w4(m2)        # PE last
            if 0 <= m3 < NMM and m3 % LM == LM - 1:
                stage_store(m3 // LM)

        # epilogue: macro-tile 72 (rows 72*476 .. 73*476), non-pipelined
        m = NMM
        r0 = m * RM
        xin = xin_pool.tile([R, S * CIN], f32, tag="xin_e")
        nc.sync.dma_start(
            out=xin[:].rearrange("p (q c) -> p q c", c=CIN),
            in_=x_d[r0:r0 + RM, :].rearrange("(q p) c -> p q c", p=R),
        )
        xbf = xbf_pool.tile([R, S * CIN], bf16, tag="xbf_e")
        nc.gpsimd.tensor_copy(xbf[:], xin[:])
        psT = psT_pool.tile([CIN, S * CPAD], f32, tag="psT")
        for s in range(S):
            nc.tensor.matmul(
                psT[:, s * CPAD:(s + 1) * CPAD],
                lhsT=xbf[:, s * CIN:(s + 1) * CIN], rhs=mixI_sb[:],
                start=True, stop=True,
            )
        xmxt = xmxt_pool.tile([CIN, S * CPAD], bf16, tag="xmxt")
        nc.vector.tensor_copy(xmxt[:, 0:2 * CPAD], psT[:, 0:2 * CPAD])
        nc.scalar.copy(xmxt[:, 2 * CPAD:], psT[:, 2 * CPAD:])
        xt = xmxt[:].rearrange("p (s q) -> p s q", q=CPAD)[:, :, R:2 * R]
        psH = psH_pool.tile([MID, RM], f32, tag="psH")
        nc.tensor.matmul(psH[:], lhsT=w2t_sb[:], rhs=xt, start=True, stop=True)
        hbf = h_pool.tile([MID, RM], bf16, tag="hbf")
        leaky(hbf, psH)
        psY2 = psY2_pool.tile([R, S * COUT], f32, tag="psY2")
        for s in range(S):
            nc.tensor.matmul(
                psY2[:, s * COUT:(s + 1) * COUT],
                lhsT=hbf[:, s * R:(s + 1) * R], rhs=w4t_sb[:],
                start=True, stop=True,
            )
        y2e = y2e_tiles[m % 2]
        nc.scalar.copy(y2e[0:R, :], psY2[:])
        psO = psO_pool.tile([R, S * COUT], f32, tag="psO")
        for s in range(S):
            nc.tensor.matmul(
                psO[:, s * COUT:(s + 1) * COUT],
                lhsT=xmxt[:, s * CPAD:s * CPAD + R], rhs=w1_sb[:],
                start=True, stop=False, skip_group_check=True,
            )
            nc.tensor.matmul(
                psO[:, s * COUT:(s + 1) * COUT],
                lhsT=mix2e_sb[:], rhs=y2e[:, s * COUT:(s + 1) * COUT],
                start=False, stop=True, skip_group_check=True,
            )
        out_sb = xin_pool.tile([R, S * CIN], f32, tag="oute")
        nc.vector.tensor_add(out_sb[:], psO[:], xin[:])
        nc.sync.dma_start(
            out=o_d[r0:r0 + RM, :].rearrange("(q p) c -> p q c", p=R),
            in_=out_sb[:].rearrange("p (q c) -> p q c", c=CIN),
        )

        # tail: last 68 rows (4 batches)
        r0 = NM * RM
        xin = xin_pool.tile([RT, CIN], f32, tag="xin_t")
        nc.sync.dma_start(out=xin[:], in_=x_d[r0:r0 + RT, :])
        xbf = xbf_pool.tile([RT, CIN], bf16, tag="xbf_t")
        nc.gpsimd.tensor_copy(xbf[:], xin[:])
        psT = psT_pool.tile([CIN, 2 * RT], f32, tag="psT")
        nc.tensor.matmul(psT[:], lhsT=xbf[:], rhs=mixIt_sb[:], start=True, stop=True)
        xm = xmxt_pool.tile([CIN, RT], bf16, tag="xm_t")
        nc.vector.tensor_copy(xm[:], psT[:, 0:RT])
        xt = xmxt_pool.tile([CIN, RT], bf16, tag="xt_t")
        nc.scalar.copy(xt[:], psT[:, RT:2 * RT])
        psH = psH_pool.tile([MID, RT], f32, tag="psH")
        nc.tensor.matmul(psH[:], lhsT=w2t_sb[:], rhs=xt[:], start=True, stop=True)
        hbf = h_pool.tile([MID, RT], bf16, tag="hbf_t")
        leaky(hbf, psH)
        psY2 = psY2_pool.tile([RT, COUT], f32, tag="psY2")
        nc.tensor.matmul(psY2[:], lhsT=hbf[:], rhs=w4t_sb[:], start=True, stop=True)
        nc.scalar.copy(y2et[0:RT, :], psY2[:])
        psO = psO_pool.tile([RT, COUT], f32, tag="psO")
        nc.tensor.matmul(psO[:], lhsT=xm[:], rhs=w1_sb[:],
                         start=True, stop=False, skip_group_check=True)
        nc.tensor.matmul(psO[:], lhsT=mix2et_sb[:], rhs=y2et[:],
                         start=False, stop=True, skip_group_check=True)
        out_sb = xin_pool.tile([RT, CIN], f32, tag="out_t")
        nc.vector.tensor_add(out_sb[:], psO[:], xin[:])
        nc.sync.dma_start(out=o_d[r0:r0 + RT, :], in_=out_sb[:])

    nc.compile()
    return nc


def _host_consts(inputs):
    bf = ml_dtypes.bfloat16
    M = _gcn_matrix(np.asarray(inputs["edge_index"]), np.asarray(inputs["edge_weight"]))
    adj = np.asarray(inputs["adj"], np.float32)
    mixI, mix2e = _mix_consts(M, adj, G)
    mixIt, mix2et = _mix_consts(M, adj, GT)
    W1 = np.asarray(inputs["W1"], np.float32)
    W2 = np.asarray(inputs["W2"], np.float32)
    W4 = np.asarray(inputs["W4"], np.float32)
    b1 = np.asarray(inputs["b1"], np.float32)
    b2 = np.asarray(inputs["b2"], np.float32)
    b4 = np.asarray(inputs["b4"], np.float32)
    b1b4 = np.stack([np.tile(b1, S), np.tile(b4, S)])
    mixI_p = np.zeros((R, CPAD), np.float32)
    mixI_p[:, :2 * R] = mixI
    return {
        "mixI": mixI_p.astype(bf),
        "mix2e": mix2e.astype(bf),
        "mixIt": mixIt.astype(bf),
        "mix2et": mix2et.astype(bf),
        "w1": np.ascontiguousarray(W1).astype(bf),
        "w2t": np.ascontiguousarray(W2.T).astype(bf),
        "w4t": np.ascontiguousarray(W4.T).astype(bf),
        "b2": np.ascontiguousarray(b2[:, None]),
        "ab2": np.ascontiguousarray(SLOPE * b2[:, None]),
        "b1b4": b1b4.astype(bf),
    }


def kernel(**inputs) -> np.ndarray:
    from concourse.bass_utils import run_bass_kernel_spmd

    if "nc" not in _CACHE:
        _CACHE["nc"] = _build_bass()
    nc = _CACHE["nc"]

    consts = _host_consts(inputs)
    vector = np.ascontiguousarray(np.asarray(inputs["vector"], np.float32))
    in_maps = []
    for c in range(N_CORES):
        m = dict(consts)
        m["x"] = np.ascontiguousarray(
            vector[c * BC:(c + 1) * BC].reshape(ROWS, CIN)
        )
        in_maps.append(m)

    res = run_bass_kernel_spmd(nc, in_maps, core_ids=list(range(N_CORES)))
    outs = [res.results[c]["out"].reshape(BC, J, CIN) for c in range(N_CORES)]
    return np.concatenate(outs, axis=0)
